# revision 28
# baseline (speedup 1.0000x reference)
"""CIN (Compressed Interaction Network) forward kernel for Trainium2.

Data-parallel over 8 NeuronCores: batch dim B=2048 is sharded 256/core,
conv weights are replicated. No cross-device communication.

Per-core layout: everything lives as (channels, n) where n = (b_local, d)
flattened to 8192 columns, processed in chunks of NC columns.

For layer l with hidden H(l) channels, the conv contraction index is
c = h*39 + f. Per K-tile we need rhs[(h,f), n] = hidden[h,n] * x0t[f,n]:
  - T tiles: x0t rows broadcast to 128 partitions via DMA from HBM
    (stride-0 source AP). The DMA engines are otherwise idle.
  - One fp16 tensor_tensor per 13-f slice on DVE (2x perf mode); the
    hidden factor is read through a stride-0 free-dim broadcast AP.
Matmuls run fp16 (1 cycle/row) with K=128 tiles; layer 0 packs the
39x39=1521 interaction rows exactly into 13 K-tiles of 117 rows.
ReLU + bias + sum-over-D are fused into ScalarE activation ops with
accum_out writing final output columns directly.
"""

import sys

if "/opt/trn_rl_repo" not in sys.path:
    sys.path.insert(0, "/opt/trn_rl_repo")

from contextlib import ExitStack

import numpy as np

import concourse.bacc as bacc
import concourse.bass as bass
import concourse.mybir as mybir
import concourse.tile as tile
from concourse import bass_utils

# Problem shapes (hardcoded per contest rules)
B, F, D = 2048, 39, 32
O = 256          # conv output channels per layer
H = 128          # hidden channels fed to layers 1,2
NCORES = 8
B_LOC = B // NCORES          # 256 batches per core
N_LOC = B_LOC * D            # 8192 columns per core

NC = 256                     # columns per chunk
NB = NC // D                 # batches per chunk (8)
KG = 13                      # f-group (K-tile slice) size
NS = F // KG                 # 3 slices
# layer-0 symmetry folding: x0 (x) x0 is symmetric, keep pairs h <= f only
NPAIR = F * (F + 1) // 2     # 780
QG = (NPAIR + 127) // 128    # 7 K-tiles
Q = QG * 128                 # 896 padded rows

F16 = mybir.dt.float16
F32 = mybir.dt.float32
AF = mybir.ActivationFunctionType

TRACE = False                # set True from test harness to profile
_LAST_RESULTS = None         # BassKernelResults of last run (for test.py)


def build_module(b_loc=B_LOC, nc_cols=NC):
    """Build the Bass/Tile module for one core (shapes are per-core)."""
    n_loc = b_loc * D
    nchunk = n_loc // nc_cols
    nb = nc_cols // D
    assert n_loc % nc_cols == 0 and nc_cols % D == 0

    nc = bacc.Bacc("TRN2", target_bir_lowering=False, debug=False)

    xtc = nc.dram_tensor("xtc", (nchunk, F, nc_cols), F16, kind="ExternalInput").ap()
    ab0 = nc.dram_tensor("ab0", (nchunk, 2, 128, QG * nc_cols), F16, kind="ExternalInput").ap()
    wt0 = nc.dram_tensor("wt0", (128, QG * O), F16, kind="ExternalInput").ap()
    wt1 = nc.dram_tensor("wt1", (128, F * O), F16, kind="ExternalInput").ap()
    wt2 = nc.dram_tensor("wt2", (128, F * O), F16, kind="ExternalInput").ap()
    biases = nc.dram_tensor("biases", (128, 8), F32, kind="ExternalInput").ap()
    out = nc.dram_tensor("out", (4, 128, b_loc), F32, kind="ExternalOutput").ap()

    with tile.TileContext(nc) as tc, ExitStack() as ctx:
        const = ctx.enter_context(tc.tile_pool(name="const", bufs=1))
        t_pool = ctx.enter_context(tc.tile_pool(name="tpool", bufs=9))
        rhs_pool = ctx.enter_context(tc.tile_pool(name="rhspool", bufs=10))
        hid_pool = ctx.enter_context(tc.tile_pool(name="hidpool", bufs=4))
        d_pool = ctx.enter_context(tc.tile_pool(name="dpool", bufs=4))
        psum_pool = ctx.enter_context(tc.tile_pool(name="psum", bufs=8, space="PSUM"))

        # --- resident tensors ---
        wt0_sb = const.tile([128, QG, O], F16)
        wt1_sb = const.tile([128, F, O], F16)
        wt2_sb = const.tile([128, F, O], F16)
        bias_sb = const.tile([128, 8], F32)
        out_sb = [const.tile([128, b_loc], F32, name=f"osb{i}") for i in range(4)]

        # Preamble DMAs: only what chunk 0's layer 0 needs, in consumption
        # order on the SP ring. wt1/wt2 are emitted lazily (per 13-f slice,
        # on the ACT HWDGE ring) right before their first consumers so the
        # startup isn't HBM-bound on 6 MB of weights.
        nc.sync.dma_start(bias_sb[:], biases)
        nc.sync.dma_start(wt0_sb[:], wt0.rearrange("p (g o) -> p g o", o=O))
        wt1_r = wt1.rearrange("p (f o) -> p f o", o=O)
        wt2_r = wt2.rearrange("p (f o) -> p f o", o=O)
        # PE warmup: dep-free matmuls over the bias tile keep the HAM
        # un-throttled through the input-load window.
        warm_ps = psum_pool.tile([128, nc_cols], F32, tag="ps", name="warm_ps")
        for _ in range(72):
            nc.tensor.matmul(
                warm_ps[0:8, 0:8],
                bias_sb[:, 0:8],
                bias_sb[:, 0:8],
                start=True,
                stop=True,
            )

        wt_sbs = [wt0_sb, wt1_sb, wt2_sb]

        def load_T(j):
            """T slices: x0t rows f in [13s,13s+13) broadcast to 128 partitions."""
            t_tiles = []
            for s in range(NS):
                t_t = t_pool.tile(
                    [128, KG, nc_cols], F16, tag="T", name=f"t_{j}_{s}", uniquify=True
                )
                src = xtc[j, s * KG : (s + 1) * KG, :]
                nc.sync.dma_start(t_t[:], src.partition_broadcast(128))
                t_tiles.append(t_t)
            return t_tiles

        def load_ab(j):
            """Host-packed folded-pair factors for layer 0 (pad rows zero)."""
            a_t = t_pool.tile([128, QG, nc_cols], F16, tag="ab", bufs=4, name=f"a_{j}")
            b_t = t_pool.tile([128, QG, nc_cols], F16, tag="ab", bufs=4, name=f"b_{j}")
            nc.sync.dma_start(a_t[:], ab0[j, 0].rearrange("p (g i) -> p g i", i=nc_cols))
            nc.sync.dma_start(b_t[:], ab0[j, 1].rearrange("p (g i) -> p g i", i=nc_cols))
            return a_t, b_t

        def build_rhs0(j, ab_t):
            rhs0 = rhs_pool.tile([128, QG, nc_cols], F16, tag="rhs", name=f"rhs_{j}_0")
            nc.vector.tensor_mul(rhs0[:], ab_t[0][:], ab_t[1][:])
            return rhs0

        def emit_mms(j, l, m, ps, rhs0, rhs_slices):
            wt_sb = wt_sbs[l]
            if l == 0:
                for g in range(QG):
                    nc.tensor.matmul(
                        ps[:],
                        wt0_sb[:, g, m * 128 : (m + 1) * 128],
                        rhs0[:, g, :],
                        start=(g == 0),
                        stop=(g == QG - 1),
                    )
            else:
                for s in range(NS):
                    for t in range(KG):
                        f = s * KG + t
                        nc.tensor.matmul(
                            ps[:],
                            wt_sb[:, f, m * 128 : (m + 1) * 128],
                            rhs_slices[s][:, t, :],
                            start=(f == 0),
                            stop=(f == F - 1),
                        )

        def direct_out(j, l, ps, bias_col, osb):
            # one full-width relu+bias on ScalarE, one DVE segment-reduce over D
            dt = d_pool.tile([128, nc_cols], F16, tag="dt", name=f"dt_{j}_{l}")
            nc.scalar.activation(
                dt[:], ps[:], AF.Relu, bias=bias_sb[:, bias_col : bias_col + 1]
            )
            nc.vector.tensor_reduce(
                osb[:, j * nb : (j + 1) * nb],
                dt[:].rearrange("p (b d) -> p b d", d=D),
                axis=mybir.AxisListType.X,
                op=mybir.AluOpType.add,
            )

        def tt_slices(j, l, newhid, t_tiles):
            new_slices = []
            for s in range(NS):
                r_t = rhs_pool.tile(
                    [128, KG, nc_cols], F16, tag="rhs", name=f"rhs_{j}_{l}_{s}"
                )
                in0b = newhid[:].unsqueeze(1).broadcast_to((128, KG, nc_cols))
                nc.vector.tensor_mul(r_t[:], in0b, t_tiles[s][:])
                new_slices.append(r_t)
            return new_slices

        def l0_block(j, rhs0, t_tiles):
            """Emit L0(j) matmuls + hidden ACT + TT_L1(j) + direct epilogue."""
            ps1 = psum_pool.tile([128, nc_cols], F32, tag="ps", name=f"ps_{j}_0_1")
            emit_mms(j, 0, 1, ps1, rhs0, None)
            h0 = hid_pool.tile([128, nc_cols], F16, tag="hid", name=f"hid_{j}_0")
            nc.scalar.activation(h0[:], ps1[:], AF.Relu, bias=bias_sb[:, 1:2])
            ps0 = psum_pool.tile([128, nc_cols], F32, tag="ps", name=f"ps_{j}_0_0")
            emit_mms(j, 0, 0, ps0, rhs0, None)
            sl1 = tt_slices(j, 1, h0, t_tiles)
            direct_out(j, 0, ps0, 0, out_sb[0])
            return sl1

        # Rotated software pipeline. Steady-state PE stream per iteration k:
        #   L1m1(k) L1m0(k) | L0m1(k+1) L0m0(k+1) | L2m0(k) L2m1(k)
        # L2(k) sits between L0(k+1) and L1(k+1), so every ACT(hidden)+TT
        # chain has ~9us of independent matmuls to hide behind.
        ab_cur = load_ab(0)
        t_prev = load_T(0)
        rhs0_cur = build_rhs0(0, ab_cur)
        sl1_cur = l0_block(0, rhs0_cur, t_prev)

        for k in range(nchunk):
            if k == 0:
                for s in range(NS):
                    nc.scalar.dma_start(
                        wt1_sb[:, s * KG : (s + 1) * KG, :],
                        wt1_r[:, s * KG : (s + 1) * KG, :],
                    )
                for s in range(NS):
                    nc.scalar.dma_start(
                        wt2_sb[:, s * KG : (s + 1) * KG, :],
                        wt2_r[:, s * KG : (s + 1) * KG, :],
                    )
            # prefetch chunk k+1 inputs and its L0 interactions
            if k + 1 < nchunk:
                ab_cur = load_ab(k + 1)
                t_cur = load_T(k + 1)
                rhs0_cur = build_rhs0(k + 1, ab_cur)

            # L1(k)
            ps1 = psum_pool.tile([128, nc_cols], F32, tag="ps", name=f"ps_{k}_1_1")
            emit_mms(k, 1, 1, ps1, None, sl1_cur)
            h1 = hid_pool.tile([128, nc_cols], F16, tag="hid", name=f"hid_{k}_1")
            nc.scalar.activation(h1[:], ps1[:], AF.Relu, bias=bias_sb[:, 3:4])
            ps0 = psum_pool.tile([128, nc_cols], F32, tag="ps", name=f"ps_{k}_1_0")
            emit_mms(k, 1, 0, ps0, None, sl1_cur)
            sl2 = tt_slices(k, 2, h1, t_prev)
            direct_out(k, 1, ps0, 2, out_sb[1])

            # L0(k+1) between L1(k) and L2(k)
            if k + 1 < nchunk:
                sl1_cur = l0_block(k + 1, rhs0_cur, t_cur)

            # L2(k)
            ps20 = psum_pool.tile([128, nc_cols], F32, tag="ps", name=f"ps_{k}_2_0")
            emit_mms(k, 2, 0, ps20, None, sl2)
            ps21 = psum_pool.tile([128, nc_cols], F32, tag="ps", name=f"ps_{k}_2_1")
            emit_mms(k, 2, 1, ps21, None, sl2)
            direct_out(k, 2, ps20, 4, out_sb[2])
            direct_out(k, 2, ps21, 5, out_sb[3])

            t_prev = t_cur if k + 1 < nchunk else None

        for i in range(4):
            nc.sync.dma_start(out[i], out_sb[i][:])

    nc.compile()
    return nc


def _pack_inputs(field_embeddings, w0, b0, w1, b1, w2, b2, b_loc=B_LOC, nc_cols=NC):
    """Host-side packing: shard x over cores, pre-transpose/convert weights."""
    x = np.asarray(field_embeddings, dtype=np.float32)
    w0 = np.asarray(w0, dtype=np.float32)
    w1 = np.asarray(w1, dtype=np.float32)
    w2 = np.asarray(w2, dtype=np.float32)
    ncores = x.shape[0] // b_loc
    n_loc = b_loc * D
    nchunk = n_loc // nc_cols

    # wt1/wt2: [h, f*O + o] = w[o, h*39 + f]
    def pack_w(w):
        a = w.reshape(O, H, F).transpose(1, 2, 0)      # (h, f, o)
        return np.ascontiguousarray(a.reshape(H, F * O)).astype(np.float16)

    # wt0 (folded): pair q=(h<=f), row p, tile g with q = g*128+p;
    # Wf[o,q] = w0[o,h*39+f] + (h!=f)*w0[o,f*39+h]
    hq = np.array([h for f_ in range(F) for h in range(f_ + 1)])
    fq = np.array([f_ for f_ in range(F) for h in range(f_ + 1)])
    w0r = w0.reshape(O, F, F)
    wf = w0r[:, hq, fq] + np.where(hq == fq, 0.0, w0r[:, fq, hq])   # (O, NPAIR)
    wf_pad = np.zeros((O, Q), dtype=np.float32)
    wf_pad[:, :NPAIR] = wf
    wt0h = np.ascontiguousarray(
        wf_pad.reshape(O, QG, 128).transpose(2, 1, 0).reshape(128, QG * O)
    ).astype(np.float16)

    wt1h = pack_w(w1)
    wt2h = pack_w(w2)

    biash = np.zeros((128, 8), dtype=np.float32)
    for li, bvec in enumerate([b0, b1, b2]):
        bvec = np.asarray(bvec, dtype=np.float32)
        biash[:, 2 * li] = bvec[0:128]
        biash[:, 2 * li + 1] = bvec[128:256]

    in_maps = []
    for c in range(ncores):
        xc = x[c * b_loc : (c + 1) * b_loc]                  # (b_loc, F, D)
        x0t = xc.transpose(1, 0, 2).reshape(F, n_loc)        # (F, n_loc)
        xtc = x0t.reshape(F, nchunk, nc_cols).transpose(1, 0, 2)
        x0t16 = x0t.astype(np.float16)
        ab = np.zeros((2, Q, n_loc), dtype=np.float16)
        ab[0, :NPAIR] = x0t16[hq]
        ab[1, :NPAIR] = x0t16[fq]
        # device layout [j, s, p, g*nc+i] with pair row q = g*128+p
        ab0 = ab.reshape(2, QG, 128, nchunk, nc_cols).transpose(3, 0, 2, 1, 4)
        ab0 = ab0.reshape(nchunk, 2, 128, QG * nc_cols)
        in_maps.append(
            {
                "xtc": np.ascontiguousarray(xtc).astype(np.float16),
                "ab0": np.ascontiguousarray(ab0),
                "wt0": wt0h,
                "wt1": wt1h,
                "wt2": wt2h,
                "biases": biash,
            }
        )
    return in_maps


_MODULE = None


def kernel(field_embeddings, w0, b0, w1, b1, w2, b2):
    global _MODULE, _LAST_RESULTS
    if _MODULE is None:
        _MODULE = build_module()
    nc = _MODULE
    in_maps = _pack_inputs(field_embeddings, w0, b0, w1, b1, w2, b2)
    res = bass_utils.run_bass_kernel_spmd(
        nc, in_maps, core_ids=list(range(NCORES)), trace=TRACE
    )
    _LAST_RESULTS = res
    outs = []
    for c in range(NCORES):
        o = res.results[c]["out"]                  # (4, 128, B_LOC) fp32
        full = o.reshape(512, B_LOC)               # [L0;L1;L2a;L2b]
        outs.append(full.T)                        # (B_LOC, 512)
    return np.ascontiguousarray(np.concatenate(outs, axis=0), dtype=np.float32)
